# revision 1
# baseline (speedup 1.0000x reference)
"""CPC loss kernel for Trainium2, data-parallel over 8 NeuronCores.

Math
----
Reference (per row x of shape [C], target t, y = x[t], C = 128):
  ce   = logsumexp(x) - y
  bdc  = -(sum_{j != t} log_sigmoid(y - x_j)) / (C-1)
  bec  = -(0.5 * sum_{j,k in rest} log_sigmoid(x_j - x_k + EPS)) / ((C-1)(C-2))

With log_sigmoid(d) = -softplus(-d), extending the rest-pair sums to the full
C x C square plus O(C) corrections (EPS = 1e-10 is invisible in fp32):

  P1 = sum_j sp(x_j - y),  P2 = sum_j sp(y - x_j)     # full C each
  SP = sum_{j,k in C x C} sp(x_j - x_k)               # diagonal included
  row_loss = (mx + ln(sum e^{x-mx}) - y)
           + (P1 - log2)/(C-1) + 0.5*(SP - P1 - P2 + log2)/((C-1)(C-2))

The ACT tables in this toolchain have no softplus, so sp comes from
Exp + Ln(bias=1) (one table set: natural_log_exp_and_others), and the pair
count is halved with sp(d) + sp(-d) = 2*ln(1+e^d) - d:

  SP = 2*LNS - LC,   LNS = sum_{j<k} ln(1+e^{d_jk}) + npad*ln2  (measured,
       with npad = 64 zero pad columns; diagonal C*ln2 = 2*npad*ln2 cancels)
  LC = sum_i x_i * (C-1-2i)          # linear, on VectorE

Kernel structure (per core, 2048 rows as 16 batches of 128):
  - TensorE: D[r, f] = x_r,j(f) - x_r,k(f) over the 8128 j<k pairs (+64 pad)
    via lhsT = X^T (bf16) times constant W[kappa, f] = delta(kappa, j(f)) -
    delta(kappa, k(f)), into [128, 2048] PSUM chunks.
  - ScalarE: Exp then Ln(bias=1, accum_out) in-place on each PSUM chunk;
    P1/P2 via the per-partition bias port (bias = -y / +y); CE sumexp.
  - VectorE: max, target gather (iota == t mask), LC, final combine.
Per-row losses are DMA'd out; the host sums across rows and cores.
"""

import functools

import numpy as np
import ml_dtypes

import concourse.bass as bass
import concourse.tile as tile
import concourse.hw_specs as hw_specs
from concourse import bacc, mybir
from concourse.bass_utils import run_bass_kernel_spmd

# The act-table chooser greedily picks the first set containing each
# function, so an exp/ln-alternating kernel loads exp_and_others and
# natural_log in alternation (~2.7us per load, ~96 loads here). Blank the
# single-function sets (keeping dict order, so act_func_set_id indices into
# act_info.json stay valid) so both exp and ln resolve to
# natural_log_exp_and_others and a single load suffices.
_orig_get_activation_tables = hw_specs.get_activation_tables


@functools.cache
def _patched_activation_tables(module_arch: str):
    d = dict(_orig_get_activation_tables(module_arch))
    for name in ("exp_and_others", "natural_log", "exp_and_friends"):
        if name in d:
            d[name] = set()
    return d


hw_specs.get_activation_tables = _patched_activation_tables
bacc.get_activation_tables = _patched_activation_tables

N, C = 16384, 128
NCORES = 8
ROWS = N // NCORES            # rows per core
P = 128                       # partitions / rows per batch
NB = ROWS // P                # batches per core
NPAIR = (C * (C - 1)) // 2    # 8128
NPAD = 64
NF = NPAIR + NPAD             # 8192 pair columns
CHUNK = 2048                  # free elems per ACT instruction (4 PSUM banks)
NCHUNK = NF // CHUNK          # 4 chunks per batch
MM_N = 512                    # moving free dim per matmul (1 PSUM bank)

F32 = mybir.dt.float32
BF16 = mybir.dt.bfloat16
AF = mybir.ActivationFunctionType
ALU = mybir.AluOpType

LOG2 = float(np.log(2.0))
C_SP = 0.5 / ((C - 1) * (C - 2))          # "c"
# row_loss = ce - (sum_{j!=t} ls(y-x_j))/(C-1) - 0.5*T/((C-1)(C-2)) with
# ce computed as ln(sum_j e^{x_j - y}) (y-shifted logsumexp: x-y <= ~10 for
# randn inputs so no overflow, and the same e^{x-y} feeds P1), P2 recovered
# from P2 = P1 - S + C*Y (S = sum_j x_j, the sp(d)+sp(-d) identity):
# row_loss = LSE + K_Y*Y + K_P1*P1 + K_S*S + 2c*LNS - c*LC + C_CONST
K_Y = -C * C_SP
K_P1 = 1.0 / (C - 1) - 2.0 * C_SP
K_S = C_SP
C_CONST = -LOG2 / (C - 1) + 0.5 * LOG2 / ((C - 1) * (C - 2))

# Number of the 4 pair-chunks per batch whose ln-sum goes through the
# VectorE group-product path (sum ln(1+u) = sum over groups of ln prod(1+u),
# groups of 8 so fp32 can't overflow) instead of a full-width ACT Ln pass.
# Balances the ScalarE (sole exp/ln engine) against the otherwise idle DVE.
N_PROD_DEFAULT = 4

_cache: dict = {}


def _build_program(repeat: int = 1, n_prod: int = N_PROD_DEFAULT) -> bass.Bass:
    # Bacc (not raw Bass): its compile() runs generate_event_semaphores,
    # which splits multi-sem waits (the ACT ISA has a single wait slot).
    nc = bacc.Bacc("TRN2")

    x_d = nc.declare_dram_parameter("x", [ROWS, C], F32, isOutput=False)
    xt_d = nc.declare_dram_parameter("xt", [C, ROWS], BF16, isOutput=False)
    w_d = nc.declare_dram_parameter("w", [C, NF], BF16, isOutput=False)
    io_d = nc.declare_dram_parameter("io", [P, C], F32, isOutput=False)
    cf_d = nc.declare_dram_parameter("cf", [P, C], F32, isOutput=False)
    tf_d = nc.declare_dram_parameter("tf", [ROWS], F32, isOutput=False)
    out_d = nc.declare_dram_parameter("out", [ROWS], F32, isOutput=True)

    with tile.TileContext(nc) as tc:
        with (
            tc.tile_pool(name="const", bufs=1) as const_pool,
            tc.tile_pool(name="work", bufs=3) as work,
            tc.tile_pool(name="acc", bufs=1) as acc_pool,
            tc.tile_pool(name="psum", bufs=2, space="PSUM") as psum_pool,
        ):
            # load order: small tensors and xt first so batch-0 work can
            # start while the 2MB W streams in (in chunk-sized pieces)
            io_sb = const_pool.tile([P, C], F32)
            nc.sync.dma_start(out=io_sb, in_=io_d[:])
            cf_sb = const_pool.tile([P, C], F32)
            nc.sync.dma_start(out=cf_sb, in_=cf_d[:])
            t_sb = const_pool.tile([P, NB], F32)
            nc.sync.dma_start(out=t_sb, in_=tf_d.rearrange("(b p) -> p b", p=P))
            xt_sb = const_pool.tile([C, ROWS], BF16)
            nc.sync.dma_start(out=xt_sb, in_=xt_d[:])
            x_sb = const_pool.tile([P, NB, C], F32)
            nc.sync.dma_start(out=x_sb, in_=x_d.rearrange("(b p) c -> p b c", p=P))
            w_sb = const_pool.tile([C, NF], BF16)
            for ch in range(NCHUNK):
                nc.sync.dma_start(
                    out=w_sb[:, ch * CHUNK : (ch + 1) * CHUNK],
                    in_=w_d[:, ch * CHUNK : (ch + 1) * CHUNK],
                )

            LNS = acc_pool.tile([P, NB], F32)
            LC = acc_pool.tile([P, NB], F32)
            P1 = acc_pool.tile([P, NB], F32)
            SU = acc_pool.tile([P, NB], F32)
            SE = acc_pool.tile([P, NB], F32)
            Y = acc_pool.tile([P, NB], F32)
            NY = acc_pool.tile([P, NB], F32)

            for _rep in range(repeat):
              for b in range(NB):
                xb = x_sb[:, b, :]
                yb = Y[:, b : b + 1]
                nyb = NY[:, b : b + 1]

                # y = x[r, t_r] via (iota == t) mask then masked row-sum
                # (tensor_tensor_reduce is a custom DVE op that dies at
                # runtime here, so use plain mul + reduce)
                mask = work.tile([P, C], F32, tag="mask")
                nc.vector.tensor_scalar(
                    mask, io_sb, t_sb[:, b : b + 1], None, op0=ALU.is_equal
                )
                nc.vector.tensor_mul(mask, mask, xb)
                nc.vector.tensor_reduce(
                    yb, mask, axis=mybir.AxisListType.X, op=ALU.add
                )
                nc.vector.tensor_scalar_mul(nyb, yb, -1.0)

                # LC = sum_i x_i * (C-1-2i)
                prod = work.tile([P, C], F32, tag="prod")
                nc.vector.tensor_mul(prod, xb, cf_sb)
                nc.vector.tensor_reduce(
                    LC[:, b : b + 1], prod, axis=mybir.AxisListType.X, op=ALU.add
                )

                # u1 = e^{x - y} feeds both P1 (ln(1+u) via the product
                # path) and the y-shifted CE logsumexp (sum u -> ln at end)
                scr1 = work.tile([P, C], F32, tag="scr1")
                nc.scalar.activation(scr1, xb, AF.Exp, bias=nyb, scale=1.0)
                nc.vector.tensor_reduce(
                    SE[:, b : b + 1], scr1, axis=mybir.AxisListType.X, op=ALU.add
                )
                p1u = work.tile([P, C], BF16, tag="p1u")
                nc.vector.tensor_scalar_add(p1u, scr1, 1.0)
                nc.vector.tensor_mul(p1u[:, :64], p1u[:, :64], p1u[:, 64:128])
                nc.vector.tensor_mul(p1u[:, :32], p1u[:, :32], p1u[:, 32:64])
                nc.vector.tensor_mul(p1u[:, :16], p1u[:, :16], p1u[:, 16:32])
                p1scr = work.tile([P, 16], F32, tag="p1scr")
                nc.scalar.activation(
                    p1scr, p1u[:, :16], AF.Ln, bias=0.0, scale=1.0,
                    accum_out=P1[:, b : b + 1],
                )
                # S = sum_j x_j
                nc.vector.tensor_reduce(
                    SU[:, b : b + 1], xb, axis=mybir.AxisListType.X, op=ALU.add
                )

                # LNS over the 8192 pair columns
                lnacc = (
                    work.tile([P, NCHUNK], F32, tag="lnacc")
                    if n_prod < NCHUNK
                    else None
                )
                G = CHUNK // 8  # group-products per chunk
                lnin = work.tile([P, NCHUNK * G], BF16, tag="lnin")
                lhsT = xt_sb[:, b * P : (b + 1) * P]
                for ch in range(NCHUNK):
                    pt = psum_pool.tile([P, CHUNK], F32, tag="pair")
                    for m in range(CHUNK // MM_N):
                        f0 = ch * CHUNK + m * MM_N
                        nc.tensor.matmul(
                            pt[:, m * MM_N : (m + 1) * MM_N],
                            lhsT,
                            w_sb[:, f0 : f0 + MM_N],
                        )
                    if ch < n_prod:
                        # DVE product path: u -> 1+u -> products of 8 ->
                        # one short Ln per batch. Frees ScalarE, which is the
                        # bottleneck. bf16 scratch: the +1 runs in DVE 4x
                        # packed mode and the multiply tree in 2x (vs 2x/1x
                        # for f32); the rounding noise is random-sign and
                        # vanishes in the 16K-row mean.
                        eu = work.tile([P, CHUNK], BF16, tag="eu")
                        nc.scalar.activation(eu, pt, AF.Exp, bias=0.0, scale=1.0)
                        nc.vector.tensor_scalar_add(eu, eu, 1.0)
                        h = CHUNK // 2
                        nc.vector.tensor_mul(eu[:, :h], eu[:, :h], eu[:, h:])
                        nc.vector.tensor_mul(
                            eu[:, : h // 2], eu[:, : h // 2], eu[:, h // 2 : h]
                        )
                        nc.vector.tensor_mul(
                            lnin[:, ch * G : (ch + 1) * G],
                            eu[:, : h // 4],
                            eu[:, h // 4 : h // 2],
                        )
                    else:
                        nc.scalar.activation(pt, pt, AF.Exp, bias=0.0, scale=1.0)
                        nc.scalar.activation(
                            pt, pt, AF.Ln, bias=1.0, scale=1.0,
                            accum_out=lnacc[:, ch : ch + 1],
                        )
                # one Ln + accum over all product-chunk groups at once
                if n_prod > 0:
                    lnscr = work.tile([P, n_prod * G], F32, tag="lnscr")
                    nc.scalar.activation(
                        lnscr, lnin[:, : n_prod * G], AF.Ln, bias=0.0, scale=1.0,
                        accum_out=LNS[:, b : b + 1],
                    )
                else:
                    nc.vector.memset(LNS[:, b : b + 1], 0.0)
                if n_prod < NCHUNK:
                    nc.vector.tensor_reduce(
                        lnacc[:, 0:1], lnacc[:, n_prod:NCHUNK],
                        axis=mybir.AxisListType.X, op=ALU.add,
                    )
                    nc.vector.tensor_add(
                        LNS[:, b : b + 1], LNS[:, b : b + 1], lnacc[:, 0:1]
                    )

            LSE = acc_pool.tile([P, NB], F32)
            nc.scalar.activation(LSE, SE, AF.Ln)

            # row_loss = LSE + K_Y*Y + K_P1*P1 + K_S*S
            #          + (2*C_SP)*LNS - C_SP*LC + C_CONST
            L = acc_pool.tile([P, NB], F32)
            T1 = acc_pool.tile([P, NB], F32)
            nc.vector.tensor_scalar_mul(T1, Y, K_Y)
            nc.vector.tensor_add(L, LSE, T1)
            nc.vector.tensor_scalar_mul(T1, P1, K_P1)
            nc.vector.tensor_add(L, L, T1)
            nc.vector.tensor_scalar_mul(T1, SU, K_S)
            nc.vector.tensor_add(L, L, T1)
            nc.vector.tensor_scalar_mul(T1, LNS, 2.0 * C_SP)
            nc.vector.tensor_add(L, L, T1)
            nc.vector.tensor_scalar_mul(T1, LC, -C_SP)
            nc.vector.tensor_add(L, L, T1)
            nc.vector.tensor_scalar_add(L, L, C_CONST)

            nc.sync.dma_start(out=out_d.rearrange("(b p) -> p b", p=P), in_=L)

    nc.compile()
    return nc


def _host_constants():
    if "w" not in _cache:
        ju, ku = np.triu_indices(C, 1)
        w = np.zeros((C, NF), np.float32)
        f = np.arange(NPAIR)
        w[ju, f] = 1.0
        w[ku, f] = -1.0
        _cache["w"] = w.astype(ml_dtypes.bfloat16)
        _cache["io"] = np.broadcast_to(
            np.arange(C, dtype=np.float32), (P, C)
        ).copy()
        _cache["cf"] = np.broadcast_to(
            (C - 1 - 2 * np.arange(C)).astype(np.float32), (P, C)
        ).copy()
    return _cache["w"], _cache["io"], _cache["cf"]


def kernel(inputs: np.ndarray, targets: np.ndarray) -> np.ndarray:
    x = np.ascontiguousarray(np.asarray(inputs, dtype=np.float32))
    t = np.asarray(targets)
    assert x.shape == (N, C) and t.shape == (N,)

    if "nc" not in _cache:
        _cache["nc"] = _build_program()
    nc = _cache["nc"]
    w, io, cf = _host_constants()

    xt = np.ascontiguousarray(x.T).astype(ml_dtypes.bfloat16)
    tf = t.astype(np.float32)

    in_maps = []
    for c in range(NCORES):
        r0, r1 = c * ROWS, (c + 1) * ROWS
        in_maps.append(
            {
                "x": np.ascontiguousarray(x[r0:r1]),
                "xt": np.ascontiguousarray(xt[:, r0:r1]),
                "w": w,
                "io": io,
                "cf": cf,
                "tf": np.ascontiguousarray(tf[r0:r1]),
            }
        )

    res = run_bass_kernel_spmd(nc, in_maps, list(range(NCORES)))
    total = 0.0
    for c in range(NCORES):
        total += np.sum(res.results[c]["out"].astype(np.float64))
    return np.float32(total / N)



# revision 36
# speedup vs baseline: 1.0759x; 1.0759x over previous
"""CPC loss kernel for Trainium2, data-parallel over 8 NeuronCores.

Math (v2: pair-sum-of-exponentials — no per-pair exp on device)
----
Per row x of shape [C], target t, y = x[t], E_j = e^{x_j}, C = 128:
  ce  = LSE - y,           LSE = ln(sum_j E_j)
  bdc = (P1f - C*y - ln2)/(C-1),       P1f = sum_{all j} ln(E_j + E_t)
  bec = [2*(LNS - P1f + y + ln2) - (C-2)(S - y) + (C-1)ln2] * c2
        with LNS = sum_{j<k} ln(E_j + E_k),  S = sum_j x_j,
        c2 = 0.5/((C-1)(C-2))
using sp(a-b) + sp(b-a) = 2 ln(e^a + e^b) - a - b and
sp(x_j - y) = ln(E_j + E_t) - y.  Collected:

  row_loss = LSE + K_y*y + K_P1*P1f + K_S*S + 2*c2*LNS + CONST

The key structural win: s_f = E_j(f) + E_k(f) for all 8128 j<k pairs is a
MATMUL of E^T (bf16, host-precomputed) with a constant 0/1 pair-incidence
matrix W2[c, f] = delta(c, j(f)) + delta(c, k(f)).  The old kernel needed a
per-pair ScalarE exp (8192 elems/row-batch); now ScalarE only runs Ln, and
the ln-of-pair-sum work is split three ways per 2048-col PSUM chunk to
balance engines:
  'A': ScalarE Ln in-place on PSUM with accum_out   (the direct route)
  'D': DVE product tree (f32 PSUM -> bf16 products of 8) + short ScalarE Ln
  'P': like 'D' but tree level 1 runs on the idle GPSIMD/Pool engine
An extra W2 column of ones gives SE = sum_j E_j per row for free (LSE).

Per-row losses are DMA'd out; the host sums across rows and cores.
"""

import functools

import numpy as np
import ml_dtypes

import concourse.bass as bass
import concourse.tile as tile
import concourse.hw_specs as hw_specs
from concourse import bacc, mybir
from concourse.bass_utils import run_bass_kernel_spmd

# The act-table chooser greedily picks the first set containing each
# function; blank the single-function sets so Exp and Ln both resolve to
# natural_log_exp_and_others and a single table load suffices.
_orig_get_activation_tables = hw_specs.get_activation_tables


@functools.cache
def _patched_activation_tables(module_arch: str):
    d = dict(_orig_get_activation_tables(module_arch))
    for name in ("exp_and_others", "natural_log", "exp_and_friends"):
        if name in d:
            d[name] = set()
    return d


hw_specs.get_activation_tables = _patched_activation_tables
bacc.get_activation_tables = _patched_activation_tables

N, C = 16384, 128
NCORES = 8
ROWS = N // NCORES            # rows per core
P = 128                       # partitions / rows per batch
NB = ROWS // P                # batches per core
NPAIR = (C * (C - 1)) // 2    # 8128
SECOL = NPAIR                 # ones-column (SE) at col 8128
NF = 8192                     # 8128 pairs + SE + 63 dead cols
CHUNK = 1024                  # pair cols per PSUM chunk (2 banks)
NCHUNK = NF // CHUNK          # 8
MM_N = 512                    # moving free dim per matmul (1 PSUM bank)
WLAST = 960                   # pair cols in the last chunk (rest is SE + dead)

F32 = mybir.dt.float32
BF16 = mybir.dt.bfloat16
AF = mybir.ActivationFunctionType
ALU = mybir.AluOpType

LOG2 = float(np.log(2.0))
C2 = 0.5 / ((C - 1) * (C - 2))
K_Y = -1.0 - C / (C - 1) + C * C2
K_P1 = 1.0 / (C - 1) - 2.0 * C2
K_S = -(C - 2) * C2
C_CONST = LOG2 * (-1.0 / (C - 1) + (C + 1) * C2)

# Per-batch routes for the 8 chunks of 1024.  Verifier constraints: GPSIMD
# cannot touch PSUM, and any instruction may read at most ONE non-scalar
# input from PSUM (which kills two-operand product trees on PSUM data).
# Legal consumers that compress a chunk in one pass:
#   'A': ScalarE Ln in-place + accum_out            (1183 ns / chunk)
#   'R': DVE tensor_reduce(op=mult) over [P, e, 8]  (1235 ns / chunk)
#        -> products of 8 consecutive pair-sums, ln'd in the per-batch
#        lnin pass on ScalarE (+107 ns)
# Pool (SBUF-only) carries the P1/bdc side path.  nA=63/nR=65 balances
# ScalarE ~90 vs DVE ~90 with PE at 55.
_PATTERNS = {
    "a4": ("A", "R", "A", "R", "A", "R", "A", "R"),
    "a3": ("R", "A", "R", "A", "R", "A", "R", "R"),
}
_ROUTE_SEQ = ["a4"] * 8 + ["a3"] + ["a4"] * 7

_cache: dict = {}


def _build_program(repeat: int = 1, route_seq=None) -> bass.Bass:
    routes = [_PATTERNS[k] for k in (route_seq or _ROUTE_SEQ)]
    nc = bacc.Bacc("TRN2")

    x_d = nc.declare_dram_parameter("x", [ROWS, C], F32, isOutput=False)
    et_d = nc.declare_dram_parameter("et", [C, ROWS], BF16, isOutput=False)
    eb_d = nc.declare_dram_parameter("eb", [ROWS, C], BF16, isOutput=False)
    w_d = nc.declare_dram_parameter("w2", [C, NF], BF16, isOutput=False)
    io_d = nc.declare_dram_parameter("io", [P, C], F32, isOutput=False)
    tf_d = nc.declare_dram_parameter("tf", [ROWS], F32, isOutput=False)
    out_d = nc.declare_dram_parameter("out", [ROWS], F32, isOutput=True)

    with tile.TileContext(nc) as tc:
        with (
            tc.tile_pool(name="const", bufs=1) as const_pool,
            tc.tile_pool(name="work", bufs=4) as work,
            tc.tile_pool(name="acc", bufs=1) as acc_pool,
            tc.tile_pool(name="psum", bufs=4, space="PSUM") as psum_pool,
        ):
            # DMA order: matmul inputs (et, w2 chunk 0) first so batch-0
            # matmuls start ~2.5us in; x/eb (y-gather, P1 — off the critical
            # path) stream in behind the remaining w2 chunks
            et_sb = const_pool.tile([C, ROWS], BF16)
            nc.sync.dma_start(out=et_sb, in_=et_d[:])
            w_sb = const_pool.tile([C, NF], BF16)
            nc.sync.dma_start(out=w_sb[:, :CHUNK], in_=w_d[:, :CHUNK])
            io_sb = const_pool.tile([P, C], F32)
            nc.sync.dma_start(out=io_sb, in_=io_d[:])
            t_sb = const_pool.tile([P, NB], F32)
            nc.sync.dma_start(out=t_sb, in_=tf_d.rearrange("(b p) -> p b", p=P))
            for ch in range(1, NCHUNK):
                nc.sync.dma_start(
                    out=w_sb[:, ch * CHUNK : (ch + 1) * CHUNK],
                    in_=w_d[:, ch * CHUNK : (ch + 1) * CHUNK],
                )
            x_sb = const_pool.tile([P, NB, C], F32)
            nc.sync.dma_start(out=x_sb, in_=x_d.rearrange("(b p) c -> p b c", p=P))
            eb_sb = const_pool.tile([P, NB, C], BF16)
            nc.sync.dma_start(out=eb_sb, in_=eb_d.rearrange("(b p) c -> p b c", p=P))

            # accumulators; LNSACC slots s*NB+b: s=0..3 A-chunk accums (in
            # per-batch order of occurrence), s=4 the lnin (R routes) accum
            LNSACC = acc_pool.tile([P, 5 * NB], F32)
            Y = acc_pool.tile([P, NB], F32)
            SU = acc_pool.tile([P, NB], F32)
            SEb = acc_pool.tile([P, NB], F32)
            EY = acc_pool.tile([P, NB], F32)
            PADD = acc_pool.tile([P, NB, C], BF16)
            P1T1 = acc_pool.tile([P, NB, C // 2], BF16)
            P1T2 = acc_pool.tile([P, NB, C // 4], BF16)
            P1T3 = acc_pool.tile([P, NB, C // 8], BF16)
            P1SCR = acc_pool.tile([P, NB, C // 8], F32)
            P1F = acc_pool.tile([P, NB], F32)
            LSE = acc_pool.tile([P, NB], F32)
            L = acc_pool.tile([P, NB], F32)

            for _rep in range(repeat):
                nc.vector.memset(LNSACC[:, 3 * NB : 4 * NB], 0.0)
                lnin_pending = []  # deferred per-batch lnin Ln: (lnin, nli, b)

                def _flush_lnin():
                    lnp, nlip, bp = lnin_pending.pop(0)
                    lnscr = work.tile([P, 640], F32, tag="lnscr")
                    nc.scalar.activation(
                        lnscr[:, :nlip], lnp[:, :nlip], AF.Ln, bias=0.0,
                        scale=1.0,
                        accum_out=LNSACC[:, 4 * NB + bp : 4 * NB + bp + 1],
                    )

                # y = x[r, t_r]: (iota == t) * x, summed, one STT per batch.
                # Emitted up front: DVE is idle during the DMA ramp, and the
                # P1 halves below want Y as early as possible.
                for b in range(NB):
                    ymscr = work.tile([P, C], F32, tag="ym")
                    nc.vector.scalar_tensor_tensor(
                        ymscr, io_sb, t_sb[:, b : b + 1], x_sb[:, b, :],
                        op0=ALU.is_equal, op1=ALU.mult,
                        accum_out=Y[:, b : b + 1],
                    )
                # S = sum_j x_j, all batches in one 3D reduce (early: only
                # needs x)
                nc.vector.tensor_reduce(
                    SU, x_sb, axis=mybir.AxisListType.X, op=ALU.add
                )

                def _p1_half(h0, h1):
                    # P1f = sum_j ln(E_j + e^y) for batches [h0, h1):
                    # tensor_scalar add, products of 8, one Ln, 3D reduce
                    nc.scalar.activation(
                        EY[:, h0:h1], Y[:, h0:h1], AF.Exp, bias=0.0, scale=1.0
                    )
                    for b2 in range(h0, h1):
                        nc.gpsimd.tensor_scalar(
                            PADD[:, b2, :], eb_sb[:, b2, :], EY[:, b2 : b2 + 1],
                            None, op0=ALU.add,
                        )
                    nc.gpsimd.tensor_mul(
                        P1T1[:, h0:h1], PADD[:, h0:h1, : C // 2],
                        PADD[:, h0:h1, C // 2 :],
                    )
                    nc.gpsimd.tensor_mul(
                        P1T2[:, h0:h1], P1T1[:, h0:h1, : C // 4],
                        P1T1[:, h0:h1, C // 4 :],
                    )
                    nc.gpsimd.tensor_mul(
                        P1T3[:, h0:h1], P1T2[:, h0:h1, : C // 8],
                        P1T2[:, h0:h1, C // 8 :],
                    )
                    nc.scalar.activation(
                        P1SCR[:, h0:h1], P1T3[:, h0:h1], AF.Ln,
                        bias=0.0, scale=1.0,
                    )
                    nc.vector.tensor_reduce(
                        P1F[:, h0:h1], P1SCR[:, h0:h1],
                        axis=mybir.AxisListType.X, op=ALU.add,
                    )

                for b in range(NB):
                    lhsT = et_sb[:, b * P : (b + 1) * P]
                    lnin = work.tile([P, 640], F32, tag="lnin")
                    nli = 0
                    nslot = 0
                    for ch in range(NCHUNK):
                        route = routes[b][ch]
                        # two PSUM lanes: A-chunks (ScalarE consumer) and
                        # R-chunks (DVE consumer), 2 bufs each so
                        # produce/consume overlaps within each lane
                        pt = psum_pool.tile(
                            [P, CHUNK], F32,
                            tag=("pA" if route == "A" else "pR"), bufs=2,
                        )
                        width = WLAST if ch == NCHUNK - 1 else CHUNK
                        if route == "A":
                            # elevated priority: the A-lane is the longest
                            # serial pipeline; scheduler dawdling on it
                            # stretches the program
                            with tc.high_priority():
                                for m in range(CHUNK // MM_N):
                                    f0 = ch * CHUNK + m * MM_N
                                    nc.tensor.matmul(
                                        pt[:, m * MM_N : (m + 1) * MM_N],
                                        lhsT,
                                        w_sb[:, f0 : f0 + MM_N],
                                    )
                                slot = nslot * NB + b
                                nslot += 1
                                nc.scalar.activation(
                                    pt[:, :width], pt[:, :width], AF.Ln,
                                    bias=0.0, scale=1.0,
                                    accum_out=LNSACC[:, slot : slot + 1],
                                )
                        else:
                            for m in range(CHUNK // MM_N):
                                f0 = ch * CHUNK + m * MM_N
                                nc.tensor.matmul(
                                    pt[:, m * MM_N : (m + 1) * MM_N],
                                    lhsT,
                                    w_sb[:, f0 : f0 + MM_N],
                                )
                            # products of 8 consecutive pair-sums in one DVE
                            # reduce (single PSUM input — verifier-legal)
                            e = width // 8
                            nc.vector.tensor_reduce(
                                lnin[:, nli : nli + e],
                                pt[:, :width].rearrange(
                                    "p (g e) -> p g e", e=8
                                ),
                                axis=mybir.AxisListType.X, op=ALU.mult,
                            )
                            nli += e
                        if ch == NCHUNK - 1:
                            # raw SE column (pair cols got Ln'd or tree'd;
                            # the SE col is untouched by either route)
                            nc.vector.tensor_copy(
                                SEb[:, b : b + 1], pt[:, WLAST : WLAST + 1]
                            )
                        if ch == 2 and lnin_pending:
                            # previous batch's lnin Ln, deferred here so it
                            # doesn't block this batch's A-chunks in the
                            # in-order ScalarE queue while it waits on the
                            # previous batch's last tree tails
                            _flush_lnin()
                    lnin_pending.append((lnin, nli, b))
                    if b == 9:
                        _p1_half(0, NB // 2)
                while lnin_pending:
                    _flush_lnin()
                _p1_half(NB // 2, NB)
                nc.scalar.activation(LSE, SEb, AF.Ln, bias=0.0, scale=1.0)

                # row_loss = LSE + K_Y*y + K_P1*P1f + K_S*S + 2*C2*LNS + CONST
                nc.vector.tensor_add(L, LNSACC[:, 0:NB], LNSACC[:, NB : 2 * NB])
                nc.vector.tensor_add(L, L, LNSACC[:, 2 * NB : 3 * NB])
                nc.vector.tensor_add(L, L, LNSACC[:, 3 * NB : 4 * NB])
                nc.vector.tensor_add(L, L, LNSACC[:, 4 * NB : 5 * NB])
                nc.vector.scalar_tensor_tensor(
                    L, L, 2.0 * C2, LSE, op0=ALU.mult, op1=ALU.add
                )
                nc.vector.scalar_tensor_tensor(
                    L, Y, K_Y, L, op0=ALU.mult, op1=ALU.add
                )
                nc.vector.scalar_tensor_tensor(
                    L, P1F, K_P1, L, op0=ALU.mult, op1=ALU.add
                )
                nc.vector.scalar_tensor_tensor(
                    L, SU, K_S, L, op0=ALU.mult, op1=ALU.add
                )
                nc.vector.tensor_scalar_add(L, L, C_CONST)

            nc.sync.dma_start(out=out_d.rearrange("(b p) -> p b", p=P), in_=L)

    nc.compile()
    return nc


def _host_constants():
    if "w2" not in _cache:
        ju, ku = np.triu_indices(C, 1)
        w = np.zeros((C, NF), np.float32)
        f = np.arange(NPAIR)
        w[ju, f] = 1.0
        w[ku, f] += 1.0
        w[:, SECOL] = 1.0
        _cache["w2"] = w.astype(ml_dtypes.bfloat16)
        _cache["io"] = np.broadcast_to(
            np.arange(C, dtype=np.float32), (P, C)
        ).copy()
    return _cache["w2"], _cache["io"]


def kernel(inputs: np.ndarray, targets: np.ndarray) -> np.ndarray:
    x = np.ascontiguousarray(np.asarray(inputs, dtype=np.float32))
    t = np.asarray(targets)
    assert x.shape == (N, C) and t.shape == (N,)

    if "nc" not in _cache:
        _cache["nc"] = _build_program()
    nc = _cache["nc"]
    w2, io = _host_constants()

    e = np.exp(x).astype(ml_dtypes.bfloat16)
    et = np.ascontiguousarray(e.T)
    tf = t.astype(np.float32)

    in_maps = []
    for c in range(NCORES):
        r0, r1 = c * ROWS, (c + 1) * ROWS
        in_maps.append(
            {
                "x": np.ascontiguousarray(x[r0:r1]),
                "et": np.ascontiguousarray(et[:, r0:r1]),
                "eb": np.ascontiguousarray(e[r0:r1]),
                "w2": w2,
                "io": io,
                "tf": np.ascontiguousarray(tf[r0:r1]),
            }
        )

    res = run_bass_kernel_spmd(nc, in_maps, list(range(NCORES)))
    total = 0.0
    for c in range(NCORES):
        total += np.sum(res.results[c]["out"].astype(np.float64))
    return np.float32(total / N)


# revision 40
# speedup vs baseline: 1.5384x; 1.4298x over previous
"""CPC loss kernel for Trainium2, data-parallel over 8 NeuronCores.

Math (v2: pair-sum-of-exponentials — no per-pair exp on device)
----
Per row x of shape [C], target t, y = x[t], E_j = e^{x_j}, C = 128:
  ce  = LSE - y,           LSE = ln(sum_j E_j)
  bdc = (P1f - C*y - ln2)/(C-1),       P1f = sum_{all j} ln(E_j + E_t)
  bec = [2*(LNS - P1f + y + ln2) - (C-2)(S - y) + (C-1)ln2] * c2
        with LNS = sum_{j<k} ln(E_j + E_k),  S = sum_j x_j,
        c2 = 0.5/((C-1)(C-2))
using sp(a-b) + sp(b-a) = 2 ln(e^a + e^b) - a - b and
sp(x_j - y) = ln(E_j + E_t) - y.  Collected:

  row_loss = LSE + K_y*y + K_P1*P1f + K_S*S + 2*c2*LNS + CONST

The key structural win: s_f = E_j(f) + E_k(f) for all 8128 j<k pairs is a
MATMUL of E^T (bf16, host-precomputed) with a constant 0/1 pair-incidence
matrix W2[c, f] = delta(c, j(f)) + delta(c, k(f)).  The old kernel needed a
per-pair ScalarE exp (8192 elems/row-batch, making ScalarE the sole
bottleneck); here the exp count drops to O(C) on the host and the per-pair
transcendental is Ln, which can be split across two engines (see the route
comment above _PATTERNS).  An extra W2 column of ones gives SE = sum_j E_j
per row for free (LSE).

Per-row losses are DMA'd out; the host sums across rows and cores.
"""

import functools

import numpy as np
import ml_dtypes

import concourse.bass as bass
import concourse.tile as tile
import concourse.hw_specs as hw_specs
from concourse import bacc, mybir
from concourse.bass_utils import run_bass_kernel_spmd

# The act-table chooser greedily picks the first set containing each
# function; blank the single-function sets so Exp and Ln both resolve to
# natural_log_exp_and_others and a single table load suffices.
_orig_get_activation_tables = hw_specs.get_activation_tables


@functools.cache
def _patched_activation_tables(module_arch: str):
    d = dict(_orig_get_activation_tables(module_arch))
    for name in ("exp_and_others", "natural_log", "exp_and_friends"):
        if name in d:
            d[name] = set()
    return d


hw_specs.get_activation_tables = _patched_activation_tables
bacc.get_activation_tables = _patched_activation_tables

N, C = 16384, 128
NCORES = 8
ROWS = N // NCORES            # rows per core
P = 128                       # partitions / rows per batch
NB = ROWS // P                # batches per core
NPAIR = (C * (C - 1)) // 2    # 8128
SECOL = NPAIR                 # ones-column (SE) at col 8128
NF = 8192                     # 8128 pairs + SE + 63 dead cols
CHUNK = 1024                  # pair cols per PSUM chunk (2 banks)
NCHUNK = NF // CHUNK          # 8
MM_N = 512                    # moving free dim per matmul (1 PSUM bank)
WLAST = 960                   # pair cols in the last chunk (rest is SE + dead)

F32 = mybir.dt.float32
BF16 = mybir.dt.bfloat16
AF = mybir.ActivationFunctionType
ALU = mybir.AluOpType

LOG2 = float(np.log(2.0))
C2 = 0.5 / ((C - 1) * (C - 2))
K_Y = -1.0 - C / (C - 1) + C * C2
K_P1 = 1.0 / (C - 1) - 2.0 * C2
K_S = -(C - 2) * C2
C_CONST = LOG2 * (-1.0 / (C - 1) + (C + 1) * C2)

# Per-batch routes for the 8 chunks of 1024.  Verifier constraints: GPSIMD
# cannot touch PSUM, and any instruction may read at most ONE non-scalar
# input from PSUM (which kills two-operand product trees on PSUM data).
# Legal consumers that compress a chunk in one pass:
#   'A': ScalarE Ln in-place + accum_out            (1183 ns / chunk)
#   'R': DVE tensor_reduce(op=mult) over [P, e, 8]  (1235 ns / chunk)
#        -> products of 8 consecutive pair-sums, ln'd in the per-batch
#        lnin pass on ScalarE (+107 ns)
# Pool (SBUF-only) carries the P1/bdc side path.  nA=63/nR=65 balances
# ScalarE ~90 vs DVE ~90 with PE at 55.
_PATTERNS = {
    "a4": ("A", "R", "A", "R", "A", "R", "A", "R"),
    "a3": ("R", "A", "R", "A", "R", "A", "R", "R"),
}
_ROUTE_SEQ = ["a4"] * 8 + ["a3"] + ["a4"] * 7

_cache: dict = {}


def _build_program(repeat: int = 1, route_seq=None) -> bass.Bass:
    routes = [_PATTERNS[k] for k in (route_seq or _ROUTE_SEQ)]
    nc = bacc.Bacc("TRN2")

    x_d = nc.declare_dram_parameter("x", [ROWS, C], F32, isOutput=False)
    et_d = nc.declare_dram_parameter("et", [C, ROWS], BF16, isOutput=False)
    eb_d = nc.declare_dram_parameter("eb", [ROWS, C], BF16, isOutput=False)
    w_d = nc.declare_dram_parameter("w2", [C, NF], BF16, isOutput=False)
    io_d = nc.declare_dram_parameter("io", [P, C], F32, isOutput=False)
    tf_d = nc.declare_dram_parameter("tf", [ROWS], F32, isOutput=False)
    out_d = nc.declare_dram_parameter("out", [ROWS], F32, isOutput=True)

    with tile.TileContext(nc) as tc:
        with (
            tc.tile_pool(name="const", bufs=1) as const_pool,
            tc.tile_pool(name="work", bufs=4) as work,
            tc.tile_pool(name="acc", bufs=1) as acc_pool,
            tc.tile_pool(name="psum", bufs=4, space="PSUM") as psum_pool,
        ):
            # DMA order: matmul inputs (et, w2 chunk 0) first so batch-0
            # matmuls start ~2.5us in; x/eb (y-gather, P1 — off the critical
            # path) stream in behind the remaining w2 chunks
            et_sb = const_pool.tile([C, ROWS], BF16)
            nc.sync.dma_start(out=et_sb, in_=et_d[:])
            w_sb = const_pool.tile([C, NF], BF16)
            nc.sync.dma_start(out=w_sb[:, :CHUNK], in_=w_d[:, :CHUNK])
            io_sb = const_pool.tile([P, C], F32)
            nc.sync.dma_start(out=io_sb, in_=io_d[:])
            t_sb = const_pool.tile([P, NB], F32)
            nc.sync.dma_start(out=t_sb, in_=tf_d.rearrange("(b p) -> p b", p=P))
            for ch in range(1, NCHUNK):
                nc.sync.dma_start(
                    out=w_sb[:, ch * CHUNK : (ch + 1) * CHUNK],
                    in_=w_d[:, ch * CHUNK : (ch + 1) * CHUNK],
                )
            x_sb = const_pool.tile([P, NB, C], F32)
            nc.sync.dma_start(out=x_sb, in_=x_d.rearrange("(b p) c -> p b c", p=P))
            eb_sb = const_pool.tile([P, NB, C], BF16)
            nc.sync.dma_start(out=eb_sb, in_=eb_d.rearrange("(b p) c -> p b c", p=P))

            # accumulators; LNSACC slots s*NB+b: s=0..3 A-chunk accums (in
            # per-batch order of occurrence), s=4 the lnin (R routes) accum
            LNSACC = acc_pool.tile([P, 5 * NB], F32)
            Y = acc_pool.tile([P, NB], F32)
            SU = acc_pool.tile([P, NB], F32)
            SEb = acc_pool.tile([P, NB], F32)
            EY = acc_pool.tile([P, NB], F32)
            PADD = acc_pool.tile([P, NB, C], BF16)
            P1T1 = acc_pool.tile([P, NB, C // 2], BF16)
            P1T2 = acc_pool.tile([P, NB, C // 4], BF16)
            P1T3 = acc_pool.tile([P, NB, C // 8], BF16)
            P1SCR = acc_pool.tile([P, NB, C // 8], F32)
            P1F = acc_pool.tile([P, NB], F32)
            LSE = acc_pool.tile([P, NB], F32)
            L = acc_pool.tile([P, NB], F32)

            for _rep in range(repeat):
                nc.vector.memset(LNSACC[:, 3 * NB : 4 * NB], 0.0)
                lnin_pending = []  # deferred per-batch lnin Ln: (lnin, nli, b)

                def _flush_lnin():
                    lnp, nlip, bp = lnin_pending.pop(0)
                    lnscr = work.tile([P, 640], F32, tag="lnscr")
                    nc.scalar.activation(
                        lnscr[:, :nlip], lnp[:, :nlip], AF.Ln, bias=0.0,
                        scale=1.0,
                        accum_out=LNSACC[:, 4 * NB + bp : 4 * NB + bp + 1],
                    )

                # y = x[r, t_r]: (iota == t) * x, summed, one STT per batch.
                # Emitted up front: DVE is idle during the DMA ramp, and the
                # P1 halves below want Y as early as possible.
                for b in range(NB):
                    ymscr = work.tile([P, C], F32, tag="ym")
                    nc.vector.scalar_tensor_tensor(
                        ymscr, io_sb, t_sb[:, b : b + 1], x_sb[:, b, :],
                        op0=ALU.is_equal, op1=ALU.mult,
                        accum_out=Y[:, b : b + 1],
                    )
                # S = sum_j x_j, all batches in one 3D reduce (early: only
                # needs x)
                nc.vector.tensor_reduce(
                    SU, x_sb, axis=mybir.AxisListType.X, op=ALU.add
                )

                def _p1_half(h0, h1):
                    # P1f = sum_j ln(E_j + e^y) for batches [h0, h1):
                    # tensor_scalar add, products of 8, one Ln, 3D reduce
                    nc.scalar.activation(
                        EY[:, h0:h1], Y[:, h0:h1], AF.Exp, bias=0.0, scale=1.0
                    )
                    for b2 in range(h0, h1):
                        nc.gpsimd.tensor_scalar(
                            PADD[:, b2, :], eb_sb[:, b2, :], EY[:, b2 : b2 + 1],
                            None, op0=ALU.add,
                        )
                    nc.gpsimd.tensor_mul(
                        P1T1[:, h0:h1], PADD[:, h0:h1, : C // 2],
                        PADD[:, h0:h1, C // 2 :],
                    )
                    nc.gpsimd.tensor_mul(
                        P1T2[:, h0:h1], P1T1[:, h0:h1, : C // 4],
                        P1T1[:, h0:h1, C // 4 :],
                    )
                    nc.gpsimd.tensor_mul(
                        P1T3[:, h0:h1], P1T2[:, h0:h1, : C // 8],
                        P1T2[:, h0:h1, C // 8 :],
                    )
                    nc.scalar.activation(
                        P1SCR[:, h0:h1], P1T3[:, h0:h1], AF.Ln,
                        bias=0.0, scale=1.0,
                    )
                    nc.vector.tensor_reduce(
                        P1F[:, h0:h1], P1SCR[:, h0:h1],
                        axis=mybir.AxisListType.X, op=ALU.add,
                    )

                for b in range(NB):
                    lhsT = et_sb[:, b * P : (b + 1) * P]
                    lnin = work.tile([P, 640], F32, tag="lnin")
                    nli = 0
                    nslot = 0
                    for ch in range(NCHUNK):
                        route = routes[b][ch]
                        # two PSUM lanes: A-chunks (ScalarE consumer) and
                        # R-chunks (DVE consumer), 2 bufs each so
                        # produce/consume overlaps within each lane
                        pt = psum_pool.tile(
                            [P, CHUNK], F32,
                            tag=("pA" if route == "A" else "pR"), bufs=2,
                        )
                        width = WLAST if ch == NCHUNK - 1 else CHUNK
                        if route == "A":
                            for m in range(CHUNK // MM_N):
                                f0 = ch * CHUNK + m * MM_N
                                nc.tensor.matmul(
                                    pt[:, m * MM_N : (m + 1) * MM_N],
                                    lhsT,
                                    w_sb[:, f0 : f0 + MM_N],
                                )
                            slot = nslot * NB + b
                            nslot += 1
                            nc.scalar.activation(
                                pt[:, :width], pt[:, :width], AF.Ln,
                                bias=0.0, scale=1.0,
                                accum_out=LNSACC[:, slot : slot + 1],
                            )
                        else:
                            for m in range(CHUNK // MM_N):
                                f0 = ch * CHUNK + m * MM_N
                                nc.tensor.matmul(
                                    pt[:, m * MM_N : (m + 1) * MM_N],
                                    lhsT,
                                    w_sb[:, f0 : f0 + MM_N],
                                )
                            # products of 8 consecutive pair-sums in one DVE
                            # reduce (single PSUM input — verifier-legal)
                            e = width // 8
                            nc.vector.tensor_reduce(
                                lnin[:, nli : nli + e],
                                pt[:, :width].rearrange(
                                    "p (g e) -> p g e", e=8
                                ),
                                axis=mybir.AxisListType.X, op=ALU.mult,
                            )
                            nli += e
                        if ch == NCHUNK - 1:
                            # raw SE column (pair cols got Ln'd or tree'd;
                            # the SE col is untouched by either route)
                            nc.vector.tensor_copy(
                                SEb[:, b : b + 1], pt[:, WLAST : WLAST + 1]
                            )
                        if ch == 2 and lnin_pending:
                            # previous batch's lnin Ln, deferred here so it
                            # doesn't block this batch's A-chunks in the
                            # in-order ScalarE queue while it waits on the
                            # previous batch's last tree tails
                            _flush_lnin()
                    lnin_pending.append((lnin, nli, b))
                    if b == 9:
                        _p1_half(0, NB // 2)
                while lnin_pending:
                    _flush_lnin()
                _p1_half(NB // 2, NB)
                nc.scalar.activation(LSE, SEb, AF.Ln, bias=0.0, scale=1.0)

                # row_loss = LSE + K_Y*y + K_P1*P1f + K_S*S + 2*C2*LNS + CONST
                nc.vector.tensor_add(L, LNSACC[:, 0:NB], LNSACC[:, NB : 2 * NB])
                nc.vector.tensor_add(L, L, LNSACC[:, 2 * NB : 3 * NB])
                nc.vector.tensor_add(L, L, LNSACC[:, 3 * NB : 4 * NB])
                nc.vector.tensor_add(L, L, LNSACC[:, 4 * NB : 5 * NB])
                nc.vector.scalar_tensor_tensor(
                    L, L, 2.0 * C2, LSE, op0=ALU.mult, op1=ALU.add
                )
                nc.vector.scalar_tensor_tensor(
                    L, Y, K_Y, L, op0=ALU.mult, op1=ALU.add
                )
                nc.vector.scalar_tensor_tensor(
                    L, P1F, K_P1, L, op0=ALU.mult, op1=ALU.add
                )
                nc.vector.scalar_tensor_tensor(
                    L, SU, K_S, L, op0=ALU.mult, op1=ALU.add
                )
                nc.vector.tensor_scalar_add(L, L, C_CONST)

            nc.sync.dma_start(out=out_d.rearrange("(b p) -> p b", p=P), in_=L)

    nc.compile()
    return nc


def _host_constants():
    if "w2" not in _cache:
        ju, ku = np.triu_indices(C, 1)
        w = np.zeros((C, NF), np.float32)
        f = np.arange(NPAIR)
        w[ju, f] = 1.0
        w[ku, f] += 1.0
        w[:, SECOL] = 1.0
        _cache["w2"] = w.astype(ml_dtypes.bfloat16)
        _cache["io"] = np.broadcast_to(
            np.arange(C, dtype=np.float32), (P, C)
        ).copy()
    return _cache["w2"], _cache["io"]


def kernel(inputs: np.ndarray, targets: np.ndarray) -> np.ndarray:
    x = np.ascontiguousarray(np.asarray(inputs, dtype=np.float32))
    t = np.asarray(targets)
    assert x.shape == (N, C) and t.shape == (N,)

    if "nc" not in _cache:
        _cache["nc"] = _build_program()
    nc = _cache["nc"]
    w2, io = _host_constants()

    e = np.exp(x).astype(ml_dtypes.bfloat16)
    et = np.ascontiguousarray(e.T)
    tf = t.astype(np.float32)

    in_maps = []
    for c in range(NCORES):
        r0, r1 = c * ROWS, (c + 1) * ROWS
        in_maps.append(
            {
                "x": np.ascontiguousarray(x[r0:r1]),
                "et": np.ascontiguousarray(et[:, r0:r1]),
                "eb": np.ascontiguousarray(e[r0:r1]),
                "w2": w2,
                "io": io,
                "tf": np.ascontiguousarray(tf[r0:r1]),
            }
        )

    res = run_bass_kernel_spmd(nc, in_maps, list(range(NCORES)))
    total = 0.0
    for c in range(NCORES):
        total += np.sum(res.results[c]["out"].astype(np.float64))
    return np.float32(total / N)


# revision 45
# speedup vs baseline: 1.6206x; 1.0535x over previous
"""CPC loss kernel for Trainium2, data-parallel over 8 NeuronCores.

Math (v2: pair-sum-of-exponentials — no per-pair exp on device)
----
Per row x of shape [C], target t, y = x[t], E_j = e^{x_j}, C = 128:
  ce  = LSE - y,           LSE = ln(sum_j E_j)
  bdc = (P1f - C*y - ln2)/(C-1),       P1f = sum_{all j} ln(E_j + E_t)
  bec = [2*(LNS - P1f + y + ln2) - (C-2)(S - y) + (C-1)ln2] * c2
        with LNS = sum_{j<k} ln(E_j + E_k),  S = sum_j x_j,
        c2 = 0.5/((C-1)(C-2))
using sp(a-b) + sp(b-a) = 2 ln(e^a + e^b) - a - b and
sp(x_j - y) = ln(E_j + E_t) - y.  Collected:

  row_loss = LSE + K_y*y + K_P1*P1f + K_S*S + 2*c2*LNS + CONST

The key structural win: s_f = E_j(f) + E_k(f) for all 8128 j<k pairs is a
MATMUL of E^T (bf16, host-precomputed) with a constant 0/1 pair-incidence
matrix W2[c, f] = delta(c, j(f)) + delta(c, k(f)).  The old kernel needed a
per-pair ScalarE exp (8192 elems/row-batch, making ScalarE the sole
bottleneck); here the exp count drops to O(C) on the host and the per-pair
transcendental is Ln, which can be split across two engines (see the route
comment above _PATTERNS).  An extra W2 column of ones gives SE = sum_j E_j
per row for free (LSE).

Per-row losses are DMA'd out; the host sums across rows and cores.
"""

import functools

import numpy as np
import ml_dtypes

import concourse.bass as bass
import concourse.tile as tile
import concourse.hw_specs as hw_specs
from concourse import bacc, mybir
from concourse.bass_utils import run_bass_kernel_spmd

# The act-table chooser greedily picks the first set containing each
# function; blank the single-function sets so Exp and Ln both resolve to
# natural_log_exp_and_others and a single table load suffices.
_orig_get_activation_tables = hw_specs.get_activation_tables


@functools.cache
def _patched_activation_tables(module_arch: str):
    d = dict(_orig_get_activation_tables(module_arch))
    for name in ("exp_and_others", "natural_log", "exp_and_friends"):
        if name in d:
            d[name] = set()
    return d


hw_specs.get_activation_tables = _patched_activation_tables
bacc.get_activation_tables = _patched_activation_tables

N, C = 16384, 128
NCORES = 8
ROWS = N // NCORES            # rows per core
P = 128                       # partitions / rows per batch
NB = ROWS // P                # batches per core
NPAIR = (C * (C - 1)) // 2    # 8128
SECOL = NPAIR                 # ones-column (SE) at col 8128
NF = 8192                     # 8128 pairs + SE + 63 dead cols
CHUNK = 1024                  # pair cols per PSUM chunk (2 banks)
NCHUNK = NF // CHUNK          # 8
MM_N = 512                    # moving free dim per matmul (1 PSUM bank)
WLAST = 960                   # pair cols in the last chunk (rest is SE + dead)

F32 = mybir.dt.float32
BF16 = mybir.dt.bfloat16
AF = mybir.ActivationFunctionType
ALU = mybir.AluOpType

LOG2 = float(np.log(2.0))
C2 = 0.5 / ((C - 1) * (C - 2))
K_Y = -1.0 - C / (C - 1) + C * C2
K_P1 = 1.0 / (C - 1) - 2.0 * C2
K_S = -(C - 2) * C2
C_CONST = LOG2 * (-1.0 / (C - 1) + (C + 1) * C2)

# Per-batch routes for the 8 chunks of 1024.  Verifier constraints: GPSIMD
# cannot touch PSUM, and any instruction may read at most ONE non-scalar
# input from PSUM (which kills two-operand product trees on PSUM data).
# Legal consumers that compress a chunk in one pass:
#   'A': ScalarE Ln in-place + accum_out            (1183 ns / chunk)
#   'R': DVE tensor_reduce(op=mult) over [P, e, 8]  (1235 ns / chunk)
#        -> products of 8 consecutive pair-sums, ln'd in the per-batch
#        lnin pass on ScalarE (+107 ns)
# Pool (SBUF-only) carries the P1/bdc side path.  nA=63/nR=65 balances
# ScalarE ~90 vs DVE ~90 with PE at 55.
_PATTERNS = {
    "a4": ("A", "R", "A", "R", "A", "R", "A", "R"),
    "a3": ("R", "A", "R", "A", "R", "A", "R", "R"),
    "a5": ("A", "R", "A", "R", "A", "R", "A", "A"),  # A-heavy endgame
}
_ROUTE_SEQ = ["a4"] * 8 + ["a3"] + ["a4"] * 6 + ["a5"]

_cache: dict = {}


def _build_program(repeat: int = 1, route_seq=None) -> bass.Bass:
    routes = [_PATTERNS[k] for k in (route_seq or _ROUTE_SEQ)]
    nc = bacc.Bacc("TRN2")

    x_d = nc.declare_dram_parameter("x", [ROWS, C], F32, isOutput=False)
    et_d = nc.declare_dram_parameter("et", [C, ROWS], BF16, isOutput=False)
    eb_d = nc.declare_dram_parameter("eb", [ROWS, C], BF16, isOutput=False)
    w_d = nc.declare_dram_parameter("w2", [C, NF], BF16, isOutput=False)
    io_d = nc.declare_dram_parameter("io", [P, C], F32, isOutput=False)
    tf_d = nc.declare_dram_parameter("tf", [ROWS], F32, isOutput=False)
    out_d = nc.declare_dram_parameter("out", [ROWS], F32, isOutput=True)

    with tile.TileContext(nc) as tc:
        with (
            tc.tile_pool(name="const", bufs=1) as const_pool,
            tc.tile_pool(name="work", bufs=4) as work,
            tc.tile_pool(name="acc", bufs=1) as acc_pool,
            tc.tile_pool(name="psum", bufs=4, space="PSUM") as psum_pool,
        ):
            # DMA order: batch-0 matmul inputs first (et cols 0:128 + w2
            # chunk 0 — ~0.7us) so the first A-chunk Ln starts ~1.2us in;
            # x/eb (y-gather, P1 — off the critical path) stream in last
            et_sb = const_pool.tile([C, ROWS], BF16)
            nc.sync.dma_start(out=et_sb[:, :P], in_=et_d[:, :P])
            w_sb = const_pool.tile([C, NF], BF16)
            nc.sync.dma_start(out=w_sb[:, :CHUNK], in_=w_d[:, :CHUNK])
            nc.sync.dma_start(out=et_sb[:, P:], in_=et_d[:, P:])
            io_sb = const_pool.tile([P, C], F32)
            nc.sync.dma_start(out=io_sb, in_=io_d[:])
            t_sb = const_pool.tile([P, NB], F32)
            nc.sync.dma_start(out=t_sb, in_=tf_d.rearrange("(b p) -> p b", p=P))
            for ch in (2, 1, 4, 6, 3, 5, 7):
                nc.sync.dma_start(
                    out=w_sb[:, ch * CHUNK : (ch + 1) * CHUNK],
                    in_=w_d[:, ch * CHUNK : (ch + 1) * CHUNK],
                )
            x_sb = const_pool.tile([P, NB, C], F32)
            nc.sync.dma_start(out=x_sb, in_=x_d.rearrange("(b p) c -> p b c", p=P))
            eb_sb = const_pool.tile([P, NB, C], BF16)
            nc.sync.dma_start(out=eb_sb, in_=eb_d.rearrange("(b p) c -> p b c", p=P))

            # accumulators; LNSACC slots s*NB+b: s=0..4 A-chunk accums (in
            # per-batch order of occurrence), s=5 the lnin (R routes) accum
            LNSACC = acc_pool.tile([P, 6 * NB], F32)
            Y = acc_pool.tile([P, NB], F32)
            SU = acc_pool.tile([P, NB], F32)
            SEb = acc_pool.tile([P, NB], F32)
            EY = acc_pool.tile([P, NB], F32)
            PADD = acc_pool.tile([P, NB, C], BF16)
            P1T1 = acc_pool.tile([P, NB, C // 2], BF16)
            P1T2 = acc_pool.tile([P, NB, C // 4], BF16)
            P1T3 = acc_pool.tile([P, NB, C // 8], BF16)
            P1SCR = acc_pool.tile([P, NB, C // 8], F32)
            P1F = acc_pool.tile([P, NB], F32)
            LSE = acc_pool.tile([P, NB], F32)
            L = acc_pool.tile([P, NB], F32)

            for _rep in range(repeat):
                nc.vector.memset(LNSACC[:, 3 * NB : 5 * NB], 0.0)
                lnin_pending = []  # deferred per-batch lnin Ln: (lnin, nli, b)

                def _flush_lnin():
                    lnp, nlip, bp = lnin_pending.pop(0)
                    lnscr = work.tile([P, 640], F32, tag="lnscr")
                    nc.scalar.activation(
                        lnscr[:, :nlip], lnp[:, :nlip], AF.Ln, bias=0.0,
                        scale=1.0,
                        accum_out=LNSACC[:, 5 * NB + bp : 5 * NB + bp + 1],
                    )

                # y = x[r, t_r]: (iota == t) * x, summed, one STT per batch.
                # Emitted up front: DVE is idle during the DMA ramp, and the
                # P1 halves below want Y as early as possible.
                for b in range(NB):
                    ymscr = work.tile([P, C], F32, tag="ym")
                    nc.vector.scalar_tensor_tensor(
                        ymscr, io_sb, t_sb[:, b : b + 1], x_sb[:, b, :],
                        op0=ALU.is_equal, op1=ALU.mult,
                        accum_out=Y[:, b : b + 1],
                    )
                # S = sum_j x_j, all batches in one 3D reduce (early: only
                # needs x)
                nc.vector.tensor_reduce(
                    SU, x_sb, axis=mybir.AxisListType.X, op=ALU.add
                )

                # P1f = sum_j ln(E_j + e^y), in two stages per half so the
                # ScalarE Ln never waits on Pool's trees (the exp stage runs
                # two batches earlier, giving Pool time to finish)
                def _p1_exp(h0, h1):
                    nc.scalar.activation(
                        EY[:, h0:h1], Y[:, h0:h1], AF.Exp, bias=0.0, scale=1.0
                    )
                    for b2 in range(h0, h1):
                        nc.gpsimd.tensor_scalar(
                            PADD[:, b2, :], eb_sb[:, b2, :], EY[:, b2 : b2 + 1],
                            None, op0=ALU.add,
                        )
                    nc.gpsimd.tensor_mul(
                        P1T1[:, h0:h1], PADD[:, h0:h1, : C // 2],
                        PADD[:, h0:h1, C // 2 :],
                    )
                    nc.gpsimd.tensor_mul(
                        P1T2[:, h0:h1], P1T1[:, h0:h1, : C // 4],
                        P1T1[:, h0:h1, C // 4 :],
                    )
                    nc.gpsimd.tensor_mul(
                        P1T3[:, h0:h1], P1T2[:, h0:h1, : C // 8],
                        P1T2[:, h0:h1, C // 8 :],
                    )

                def _combine(h0, h1):
                    # row_loss = LSE + K_Y*y + K_P1*P1f + K_S*S
                    #          + 2*C2*LNS + CONST, for batches [h0, h1)
                    sl = slice(h0, h1)
                    nc.scalar.activation(
                        LSE[:, sl], SEb[:, sl], AF.Ln, bias=0.0, scale=1.0
                    )
                    Lh = L[:, sl]
                    nc.vector.tensor_add(
                        Lh, LNSACC[:, h0:h1], LNSACC[:, NB + h0 : NB + h1]
                    )
                    for s5 in range(2, 6):
                        nc.vector.tensor_add(
                            Lh, Lh, LNSACC[:, s5 * NB + h0 : s5 * NB + h1]
                        )
                    nc.vector.scalar_tensor_tensor(
                        Lh, Lh, 2.0 * C2, LSE[:, sl], op0=ALU.mult, op1=ALU.add
                    )
                    nc.vector.scalar_tensor_tensor(
                        Lh, Y[:, sl], K_Y, Lh, op0=ALU.mult, op1=ALU.add
                    )
                    nc.vector.scalar_tensor_tensor(
                        Lh, P1F[:, sl], K_P1, Lh, op0=ALU.mult, op1=ALU.add
                    )
                    nc.vector.scalar_tensor_tensor(
                        Lh, SU[:, sl], K_S, Lh, op0=ALU.mult, op1=ALU.add
                    )
                    nc.vector.tensor_scalar_add(Lh, Lh, C_CONST)

                def _p1_ln(h0, h1):
                    nc.scalar.activation(
                        P1SCR[:, h0:h1], P1T3[:, h0:h1], AF.Ln,
                        bias=0.0, scale=1.0,
                    )
                    nc.vector.tensor_reduce(
                        P1F[:, h0:h1], P1SCR[:, h0:h1],
                        axis=mybir.AxisListType.X, op=ALU.add,
                    )

                for b in range(NB):
                    lhsT = et_sb[:, b * P : (b + 1) * P]
                    lnin = work.tile([P, 640], F32, tag="lnin")
                    nli = 0
                    nslot = 0
                    for ch in range(NCHUNK):
                        route = routes[b][ch]
                        # two PSUM lanes: A-chunks (ScalarE consumer) and
                        # R-chunks (DVE consumer), 2 bufs each so
                        # produce/consume overlaps within each lane
                        pt = psum_pool.tile(
                            [P, CHUNK], F32,
                            tag=("pA" if route == "A" else "pR"), bufs=2,
                        )
                        width = WLAST if ch == NCHUNK - 1 else CHUNK
                        if route == "A":
                            for m in range(CHUNK // MM_N):
                                f0 = ch * CHUNK + m * MM_N
                                nc.tensor.matmul(
                                    pt[:, m * MM_N : (m + 1) * MM_N],
                                    lhsT,
                                    w_sb[:, f0 : f0 + MM_N],
                                )
                            slot = nslot * NB + b
                            nslot += 1
                            nc.scalar.activation(
                                pt[:, :width], pt[:, :width], AF.Ln,
                                bias=0.0, scale=1.0,
                                accum_out=LNSACC[:, slot : slot + 1],
                            )
                        else:
                            for m in range(CHUNK // MM_N):
                                f0 = ch * CHUNK + m * MM_N
                                nc.tensor.matmul(
                                    pt[:, m * MM_N : (m + 1) * MM_N],
                                    lhsT,
                                    w_sb[:, f0 : f0 + MM_N],
                                )
                            # products of 8 consecutive pair-sums in one DVE
                            # reduce (single PSUM input — verifier-legal)
                            e = width // 8
                            nc.vector.tensor_reduce(
                                lnin[:, nli : nli + e],
                                pt[:, :width].rearrange(
                                    "p (g e) -> p g e", e=8
                                ),
                                axis=mybir.AxisListType.X, op=ALU.mult,
                            )
                            nli += e
                        if ch == NCHUNK - 1:
                            # raw SE column (pair cols got Ln'd or tree'd;
                            # the SE col is untouched by either route)
                            nc.vector.tensor_copy(
                                SEb[:, b : b + 1], pt[:, WLAST : WLAST + 1]
                            )
                        if ch == 2 and lnin_pending:
                            # previous batch's lnin Ln, deferred here so it
                            # doesn't block this batch's A-chunks in the
                            # in-order ScalarE queue while it waits on the
                            # previous batch's last tree tails
                            _flush_lnin()
                    lnin_pending.append((lnin, nli, b))
                    if b == 6:
                        _p1_exp(0, NB // 2)
                    elif b == 8:
                        _p1_ln(0, NB // 2)
                    elif b == 10:
                        _p1_exp(NB // 2, NB)
                    elif b == 12:
                        _p1_ln(NB // 2, NB)
                    elif b == 13:
                        _combine(0, NB // 2)
                while lnin_pending:
                    _flush_lnin()
                _combine(NB // 2, NB)

            nc.sync.dma_start(out=out_d.rearrange("(b p) -> p b", p=P), in_=L)

    nc.compile()
    return nc


def _host_constants():
    if "w2" not in _cache:
        ju, ku = np.triu_indices(C, 1)
        w = np.zeros((C, NF), np.float32)
        f = np.arange(NPAIR)
        w[ju, f] = 1.0
        w[ku, f] += 1.0
        w[:, SECOL] = 1.0
        _cache["w2"] = w.astype(ml_dtypes.bfloat16)
        _cache["io"] = np.broadcast_to(
            np.arange(C, dtype=np.float32), (P, C)
        ).copy()
    return _cache["w2"], _cache["io"]


def kernel(inputs: np.ndarray, targets: np.ndarray) -> np.ndarray:
    x = np.ascontiguousarray(np.asarray(inputs, dtype=np.float32))
    t = np.asarray(targets)
    assert x.shape == (N, C) and t.shape == (N,)

    if "nc" not in _cache:
        _cache["nc"] = _build_program()
    nc = _cache["nc"]
    w2, io = _host_constants()

    e = np.exp(x).astype(ml_dtypes.bfloat16)
    et = np.ascontiguousarray(e.T)
    tf = t.astype(np.float32)

    in_maps = []
    for c in range(NCORES):
        r0, r1 = c * ROWS, (c + 1) * ROWS
        in_maps.append(
            {
                "x": np.ascontiguousarray(x[r0:r1]),
                "et": np.ascontiguousarray(et[:, r0:r1]),
                "eb": np.ascontiguousarray(e[r0:r1]),
                "w2": w2,
                "io": io,
                "tf": np.ascontiguousarray(tf[r0:r1]),
            }
        )

    res = run_bass_kernel_spmd(nc, in_maps, list(range(NCORES)))
    total = 0.0
    for c in range(NCORES):
        total += np.sum(res.results[c]["out"].astype(np.float64))
    return np.float32(total / N)
